# revision 6
# baseline (speedup 1.0000x reference)
"""Blended-expert MoE MLP (moe_routing) Trainium2 Bass kernel.

Math per layer l:  t[b,o] = sum_e wb[b,e] * (W_l[e] @ x[b] + B_l[e])
                   x_next = elu(t)   (layers 0,1; layer 2 linear)

Reformulated as one GEMM per layer with contraction k = (i_tile, e, p):
    t[o, b] = sum_k Wp[k, o] * xp[k, b]
where xp[(i_tile,e,p), b] = xT[i_tile*128+p, b] * wbT[e, b]  (built on-chip
by DVE) and the bias enters as an extra K=8 matmul with rhs = wbT directly.

Everything on-device is feature-major ([feature, batch]) so each layer's
PSUM output [o, b] is directly the next layer's input layout.

Sharding: data-parallel over batch: 2048 -> 8 cores x 256. Weights are
replicated (streamed from HBM each layer, ~67MB/core).
"""

import os
import sys

import numpy as np

sys.path.insert(0, "/opt/trn_rl_repo")

from concourse import bacc, mybir  # noqa: E402
import concourse.bass as bass  # noqa: E402
import concourse.tile as tile  # noqa: E402

F32 = mybir.dt.float32
F32R = mybir.dt.float32r

E = 8
DIMS = [512, 1024, 1024, 512]
BATCH = 2048
NCORES = 8
B = BATCH // NCORES  # 256 per-core batch
P = 128

NI = [DIMS[0] // P, DIMS[1] // P, DIMS[2] // P]  # [4, 8, 8] input tiles / layer
NO = [DIMS[1] // P, DIMS[2] // P, DIMS[3] // P]  # [8, 8, 4] output tiles / layer
KT = [NI[l] * E for l in range(3)]  # [32, 64, 64] contraction tiles / layer

_CACHE = {}


def _build_program(mm_f32r: bool = True):
    """Build (and cache) the Bass program. Same program runs SPMD on all cores."""
    key = ("prog", mm_f32r)
    if key in _CACHE:
        return _CACHE[key]

    nc = bacc.Bacc("TRN2", target_bir_lowering=False, debug=False, num_devices=NCORES)

    xT_d = nc.dram_tensor("xT", [DIMS[0], B], F32, kind="ExternalInput")
    wbT_d = nc.dram_tensor("wbT", [E, B], F32, kind="ExternalInput")
    wp_d = [
        nc.dram_tensor(f"Wp{l}", [KT[l] * P + E, DIMS[l + 1]], F32, kind="ExternalInput")
        for l in range(3)
    ]
    yT_d = nc.dram_tensor("yT", [DIMS[3], B], F32, kind="ExternalOutput")

    MMDT = F32R if mm_f32r else F32

    with tile.TileContext(nc) as tc:
        with (
            tc.tile_pool(name="const", bufs=1) as const_pool,
            tc.tile_pool(name="xpool", bufs=2) as x_pool,
            tc.tile_pool(name="xppool", bufs=1) as xp_pool,
            tc.tile_pool(name="wstream", bufs=6) as w_pool,
            tc.tile_pool(name="wbias", bufs=2) as wb_pool,
            tc.tile_pool(name="tmp", bufs=8) as tmp_pool,
            tc.tile_pool(name="psum", bufs=8, space="PSUM") as psum_pool,
        ):
            # ---- constants / small inputs ----
            wb_sb = const_pool.tile([E, B], MMDT)
            nc.sync.dma_start(wb_sb[:], wbT_d[:].bitcast(MMDT))

            # wb broadcast to all 128 partitions: [128, E, B]
            wb_bc = const_pool.tile([P, E, B], F32)
            nc.sync.dma_start(
                wb_bc[:],
                wbT_d.rearrange("e b -> (e b)")
                .unsqueeze(0)
                .partition_broadcast(P)
                .squeeze(1)
                .rearrange("p (e b) -> p e b", e=E),
            )

            # initial x: [128, 4, B] from xT (feature-major)
            x_sb = x_pool.tile([P, NI[0], B], F32, tag="x")
            for t in range(NI[0]):
                nc.sync.dma_start(x_sb[:, t, :], xT_d[t * P : (t + 1) * P, :])

            for l in range(3):
                nI, nO, O = NI[l], NO[l], DIMS[l + 1]

                # ---- build xp[(i,e), b] = x[i,b] * wb[e,b] (DVE) ----
                xp = xp_pool.tile([P, KT[2], B], MMDT, tag="xp")
                for it in range(nI):
                    nc.vector.tensor_tensor(
                        out=xp[:, it * E : (it + 1) * E, :],
                        in0=x_sb[:, it : it + 1, :].broadcast_to([P, E, B]),
                        in1=wb_bc[:],
                        op=mybir.AluOpType.mult,
                    )

                # ---- bias rows ----
                wbias = wb_pool.tile([E, O], MMDT, tag="wbias")
                nc.sync.dma_start(
                    wbias[:], wp_d[l][KT[l] * P : KT[l] * P + E, :].bitcast(MMDT)
                )

                # ---- PSUM accumulators, one bank per o-tile ----
                po = []
                for ot in range(nO):
                    t_ps = psum_pool.tile([P, B], F32, tag="po", name=f"po_{l}_{ot}")
                    po.append(t_ps)
                    # bias matmul opens the accumulation group (start=True)
                    nc.tensor.matmul(
                        t_ps[:],
                        wbias[:, ot * P : (ot + 1) * P],
                        wb_sb[:],
                        start=True,
                        stop=False,
                    )

                # ---- stream weights, accumulate ----
                for kt in range(KT[l]):
                    w_sb = w_pool.tile([P, O], MMDT, tag="w", name=f"w_{l}_{kt}")
                    nc.sync.dma_start(
                        w_sb[:], wp_d[l][kt * P : (kt + 1) * P, :].bitcast(MMDT)
                    )
                    last = kt == KT[l] - 1
                    for ot in range(nO):
                        nc.tensor.matmul(
                            po[ot][:],
                            w_sb[:, ot * P : (ot + 1) * P],
                            xp[:, kt, :],
                            start=False,
                            stop=last,
                        )

                # ---- evict + ELU ----
                x_next = x_pool.tile([P, max(nO, NI[0]), B], F32, tag="x", name=f"x{l + 1}")
                for ot in range(nO):
                    if l < 2:
                        # elu(t) = (min(exp(t),1) - 1) + max(t, 0)
                        ex = tmp_pool.tile([P, B], F32, tag="ex", name=f"ex_{l}_{ot}")
                        nc.scalar.activation(
                            ex[:], po[ot][:], mybir.ActivationFunctionType.Exp
                        )
                        em1 = tmp_pool.tile([P, B], F32, tag="em1", name=f"em1_{l}_{ot}")
                        nc.vector.tensor_scalar(
                            em1[:],
                            ex[:],
                            1.0,
                            -1.0,
                            op0=mybir.AluOpType.min,
                            op1=mybir.AluOpType.add,
                        )
                        nc.vector.scalar_tensor_tensor(
                            x_next[:, ot, :],
                            po[ot][:],
                            0.0,
                            em1[:],
                            op0=mybir.AluOpType.max,
                            op1=mybir.AluOpType.add,
                        )
                    else:
                        nc.vector.tensor_copy(x_next[:, ot, :], po[ot][:])
                x_sb = x_next

            # ---- store result ----
            for t in range(NO[2]):
                nc.sync.dma_start(yT_d[t * P : (t + 1) * P, :], x_sb[:, t, :])

    nc.compile()
    _CACHE[key] = nc
    return nc


def _prep_weights(W, Bias, l):
    """Rearrange (E, O, I) weights + (E, O, 1) bias into the streamed layout:
    rows kt*128+p with kt = i_tile*E + e holding W[e, :, i_tile*128+p], then
    E bias rows."""
    O, I = DIMS[l + 1], DIMS[l]
    nI = I // P
    Wt = np.ascontiguousarray(
        W.transpose(2, 0, 1).reshape(nI, P, E, O).transpose(0, 2, 1, 3).reshape(nI * E * P, O),
        dtype=np.float32,
    )
    return np.concatenate([Wt, Bias[:, :, 0].astype(np.float32)], axis=0)


def _prep_in_maps(weight_blend, x, W0, B0, W1, B1, W2, B2):
    weight_blend = np.asarray(weight_blend, dtype=np.float32)
    x = np.asarray(x, dtype=np.float32)
    Ws = [np.asarray(w, dtype=np.float32) for w in (W0, W1, W2)]
    Bs = [np.asarray(b, dtype=np.float32) for b in (B0, B1, B2)]
    wp = [_prep_weights(Ws[l], Bs[l], l) for l in range(3)]
    in_maps = []
    for c in range(NCORES):
        sl = slice(c * B, (c + 1) * B)
        in_maps.append(
            {
                "xT": np.ascontiguousarray(x[sl].T),
                "wbT": np.ascontiguousarray(weight_blend[sl].T),
                "Wp0": wp[0],
                "Wp1": wp[1],
                "Wp2": wp[2],
            }
        )
    return in_maps


def kernel(weight_blend, x, W0, B0, W1, B1, W2, B2):
    from concourse.bass_utils import run_bass_kernel_spmd

    in_maps = _prep_in_maps(weight_blend, x, W0, B0, W1, B1, W2, B2)
    nc = _build_program(mm_f32r=os.environ.get("MOE_MM_DTYPE", "f32r") == "f32r")
    res = run_bass_kernel_spmd(nc, in_maps, list(range(NCORES)))
    out = np.concatenate([res.results[c]["yT"] for c in range(NCORES)], axis=1)
    return np.ascontiguousarray(out.T, dtype=np.float32)


def _make_sharded_fn(nc):
    """Build the shard_map'd jitted executable, mirroring
    bass2jax.run_bass_via_pjrt's multi-core path but without output donation
    so it can be re-invoked for timing."""
    import jax
    from jax.experimental.shard_map import shard_map
    from jax.sharding import Mesh, PartitionSpec
    from concourse import bass2jax, mybir as _mybir

    bass2jax.install_neuronx_cc_hook()

    partition_name = nc.partition_id_tensor.name if nc.partition_id_tensor else None
    in_names, out_names, out_avals, zero_outs = [], [], [], []
    for alloc in nc.m.functions[0].allocations:
        if not isinstance(alloc, _mybir.MemoryLocationSet):
            continue
        name = alloc.memorylocations[0].name
        if alloc.kind == "ExternalInput":
            if name != partition_name:
                in_names.append(name)
        elif alloc.kind == "ExternalOutput":
            out_names.append(name)
            shape = tuple(alloc.tensor_shape)
            dtype = _mybir.dt.np(alloc.dtype)
            out_avals.append(jax.core.ShapedArray(shape, dtype))
            zero_outs.append(np.zeros(shape, dtype))
    n_params = len(in_names)
    all_names = in_names + out_names
    if partition_name is not None:
        all_names = all_names + [partition_name]

    def _body(*args):
        operands = list(args)
        if partition_name is not None:
            operands.append(bass2jax.partition_id_tensor())
        outs = bass2jax._bass_exec_p.bind(
            *operands,
            out_avals=tuple(out_avals),
            in_names=tuple(all_names),
            out_names=tuple(out_names),
            lowering_input_output_aliases=(),
            sim_require_finite=True,
            sim_require_nnan=True,
            nc=nc,
        )
        return tuple(outs)

    devices = jax.devices()[:NCORES]
    mesh = Mesh(np.asarray(devices), ("core",))
    n_all = n_params + len(out_names)
    sharded = jax.jit(
        shard_map(
            _body,
            mesh=mesh,
            in_specs=(PartitionSpec("core"),) * n_all,
            out_specs=(PartitionSpec("core"),) * len(out_names),
            check_rep=False,
        ),
        keep_unused=True,
    )
    return sharded, mesh, in_names, out_names, zero_outs


def bench(weight_blend, x, W0, B0, W1, B1, W2, B2, iters=20):
    """Run kernel with inputs pre-staged on device; time repeated executions.
    Returns (output, per_iter_seconds)."""
    import time as _time

    import jax
    from jax.sharding import NamedSharding, PartitionSpec

    in_maps = _prep_in_maps(weight_blend, x, W0, B0, W1, B1, W2, B2)
    nc = _build_program(mm_f32r=os.environ.get("MOE_MM_DTYPE", "f32r") == "f32r")
    sharded, mesh, in_names, out_names, zero_outs = _make_sharded_fn(nc)

    spec = NamedSharding(mesh, PartitionSpec("core"))
    args = []
    for name in in_names:
        concat = np.concatenate([in_maps[c][name] for c in range(NCORES)], axis=0)
        args.append(jax.device_put(concat, spec))
    for z in zero_outs:
        concat = np.concatenate([z] * NCORES, axis=0)
        args.append(jax.device_put(concat, spec))

    # warmup (includes NEFF compile on first call)
    outs = sharded(*args)
    jax.block_until_ready(outs)
    for _ in range(3):
        outs = sharded(*args)
    jax.block_until_ready(outs)

    t0 = _time.perf_counter()
    for _ in range(iters):
        outs = sharded(*args)
    jax.block_until_ready(outs)
    t1 = _time.perf_counter()
    per_iter = (t1 - t0) / iters

    yt = np.asarray(outs[out_names.index("yT")]).reshape(NCORES, DIMS[3], B)
    out = np.concatenate(list(yt), axis=1)
    return np.ascontiguousarray(out.T, dtype=np.float32), per_iter
